# revision 8
# baseline (speedup 1.0000x reference)
"""Multi-head attention (S=2048, B=2, E=1024, H=16, D=64) on 8 Trainium2 cores.

Sharding: batch*heads head-parallel. Core c owns heads {2c, 2c+1} for both
batch elements (4 of the 32 (b,h) attention pairs). Host-side sharding prep:
slice/scale/cast the in_proj weights per core, cast x to bf16 and lay it out
as xT tiles [B, 2, KT, 128, 1024] (contraction dim on partitions, tokens
split into two halves), and concatenate the per-core outputs along E.

Pipeline (v2 — ACT-paced):
  HEAD  x(b0) DMAs stream per (token-half, k-tile); k/v projections and the
        first q-chunk accumulate round-by-round as tiles land, so the first
        exp fires right after b0's x finishes loading (~20us vs ~38us when
        projecting after the full load). A PE warm-up braid of dummy matmuls
        beats the HAM cold clock before the first projection round.
  STEADY per (b, qc, kt) step: the two heads' score matmuls go to one
        [128,1024] fp32 PSUM tile as a row-tiled concurrent pair (K=64,
        lhsT base partitions 0/64); one ScalarE Exp per kt evacuates
        PSUM->SBUF bf16. The attT (attn^T accumulation) for kt runs one kt
        LATE, emitted after the next kt's score matmuls: ScalarE never waits
        on the PE, which works through sc(kt+1) | attT(kt) | pumped-proj in
        each exp's shadow. attT itself: head0 as a K-split row-tiled pair
        (kpos 0-63 / 64-127 into separate PSUM banks, concurrent), head1
        plain M=65. Row 64 of each accumulator is sum(exp) (ones columns in
        va). b1's projections + v transposes pump into PE slack at rate/kt.
  TAIL  each finished qc defers its normalize (PE transpose back to [q,d],
        reciprocal of the denominator row, per-partition scalar multiply)
        into following steps' slack; output DMA per qs-block of 128 tokens
        on alternating queues.

PSUM budget (8 banks of 2KB): sc 2x[128,1024]f32 = 4, attT accumulators
h0lo/h0hi/h1 = 3, shared transient (warm/proj/vtrans/norm-transpose) = 1.
Measured on trn2: 227us baseline -> this version targets ~175us.
"""

import numpy as np
import ml_dtypes

S, B, E = 2048, 2, 1024
H, D = 16, 64
SCALING = D ** -0.5
NCORES = 8
HPC = H // NCORES     # 2 heads per core
KT = E // 128         # 8 contraction tiles over E
QCHUNK = 512
NQC = S // QCHUNK     # 4 q-chunks
NKT = S // 128        # 16 kpos tiles
VN = 2 * (D + 1)      # 130 va cols: [v_h0(64) | 1 | v_h1(64) | 1]
TH = 2                # token halves of 1024

_BF16 = ml_dtypes.bfloat16
_BUILT = {}


def _build_bass():
    import concourse.bacc as bacc
    import concourse.mybir as mybir
    import concourse.tile as tile
    from contextlib import ExitStack

    f32 = mybir.dt.float32
    bf = mybir.dt.bfloat16

    nc = bacc.Bacc(None, target_bir_lowering=False, debug=False)

    xt_in = nc.dram_tensor("xt", [B, TH, KT, 128, 1024], bf, kind="ExternalInput")
    wqkv_in = nc.dram_tensor("wqkv", [E, 384], bf, kind="ExternalInput")
    bqkv_in = nc.dram_tensor("bqkv", [384, 1], f32, kind="ExternalInput")
    id128_in = nc.dram_tensor("id128", [128, 128], bf, kind="ExternalInput")
    id65_in = nc.dram_tensor("id65", [65, 65], f32, kind="ExternalInput")
    out_d = nc.dram_tensor("out", [S, B, 2 * D], f32, kind="ExternalOutput")

    with tile.TileContext(nc) as tc, ExitStack() as ctx:
        const = ctx.enter_context(tc.tile_pool(name="const", bufs=1))
        res = ctx.enter_context(tc.tile_pool(name="res", bufs=1))
        expp = ctx.enter_context(tc.tile_pool(name="expp", bufs=4))
        atn = ctx.enter_context(tc.tile_pool(name="atn", bufs=4))
        ogp = ctx.enter_context(tc.tile_pool(name="ogp", bufs=8))
        rp = ctx.enter_context(tc.tile_pool(name="rp", bufs=16))
        ps_sc = ctx.enter_context(tc.tile_pool(name="ps_sc", bufs=2, space="PSUM"))
        ps_att = ctx.enter_context(tc.tile_pool(name="ps_att", bufs=3, space="PSUM"))
        ps_tr = ctx.enter_context(tc.tile_pool(name="ps_tr", bufs=1, space="PSUM"))

        # ---- constants on the gpsimd queue (sync/scalar kept free for x) ----
        wqkv_sb = [const.tile([128, 384], bf, tag=f"wqkv{k}", name=f"wqkv{k}") for k in range(KT)]
        for k in range(KT):
            nc.gpsimd.dma_start(out=wqkv_sb[k][:], in_=wqkv_in[k * 128:(k + 1) * 128, :])
        bqkv_sb = const.tile([128, 3], f32, tag="bqkv")
        nc.gpsimd.dma_start(
            out=bqkv_sb[:], in_=bqkv_in.rearrange("(c p) o -> p (c o)", p=128)
        )
        id128 = const.tile([128, 128], bf, tag="id128")
        nc.gpsimd.dma_start(out=id128[:], in_=id128_in[:])
        id65 = const.tile([65, 65], f32, tag="id65")
        nc.gpsimd.dma_start(out=id65[:], in_=id65_in[:])

        # ---- x DMAs: per (b, th, k) 256KB chunks, b0 first, 2 queues ----
        xs_sb = [
            [res.tile([128, KT, 1024], bf, tag=f"xs{b}_{t}", name=f"xs{b}_{t}") for t in range(TH)]
            for b in range(B)
        ]
        for b in range(B):
            for th in range(TH):
                for k in range(KT):
                    eng = (nc.sync, nc.scalar)[k % 2]
                    eng.dma_start(out=xs_sb[b][th][:, k, :], in_=xt_in[b, th, k])

        def xs(b, t):
            # projection chunk t (512 tokens) -> (th, sub) slices, per k below
            return xs_sb[b][t // 2]

        def xk(b, t, k):
            return xs_sb[b][t // 2][:, k, (t % 2) * 512:(t % 2) * 512 + 512]

        # ---- persistent SBUF results ----
        qT = [res.tile([128, S], bf, tag=f"qT{b}", name=f"qTt{b}") for b in range(B)]
        kT = [res.tile([128, S], bf, tag=f"kT{b}", name=f"kTt{b}") for b in range(B)]
        vT = [res.tile([128, S], bf, tag=f"vT{b}", name=f"vTt{b}") for b in range(B)]
        va = [res.tile([128, NKT, VN], bf, tag=f"va{b}", name=f"vat{b}") for b in range(B)]
        for b in range(B):
            nc.vector.memset(va[b][:], 1.0)  # ones cols survive at 64, 129

        # ---- PE warm-up braid: dummy matmuls against the HAM cold clock ----
        dm = res.tile([128, 256], bf, tag="dm")
        nc.vector.memset(dm[:], 0.125)
        for _ in range(70):
            warm = ps_tr.tile([128, 256], f32, tag="tr", name="warm")
            nc.tensor.matmul(
                warm[:], lhsT=dm[:, 0:128], rhs=dm[:], start=True, stop=True
            )

        def bias_evac(ps, dst_col_slice, which):
            nc.vector.tensor_scalar_add(
                out=dst_col_slice, in0=ps[:], scalar1=bqkv_sb[:, which:which + 1]
            )

        def vtrans_unit(b, kt2):
            # one [128,128] transpose: vT cols kt2*128.. -> va[:, kt2, d-cols]
            pst = ps_tr.tile([128, 128], bf, tag="tr", name="vtps")
            nc.tensor.transpose(
                pst[:], in_=vT[b][:, kt2 * 128:(kt2 + 1) * 128], identity=id128[:]
            )
            nc.vector.tensor_copy(
                out=va[b][:, kt2, :].rearrange("p (g x) -> p g x", g=2)[:, :, 0:64],
                in_=pst[:].rearrange("p (g d) -> p g d", g=2),
            )

        # ---- HEAD: b0 k/v/q0 projections pipelined with the x stream ----
        # accumulators: k chunks 2t+0/2t+1 and v chunks in ps_att + ps_sc
        # slots, q-chunk0 in ps_tr (after the warm braid).
        kacc = {}
        vacc = {}
        qacc = None
        for th in range(TH):
            kacc[2 * th] = ps_att.tile([128, QCHUNK], f32, tag="att", name="kacc0")
            kacc[2 * th + 1] = ps_att.tile([128, QCHUNK], f32, tag="att", name="kacc1")
            vacc[2 * th] = ps_att.tile([128, QCHUNK], f32, tag="att", name="vacc0")
            vacc[2 * th + 1] = ps_sc.tile([128, QCHUNK], f32, tag="sc", name="vacc1")
            if th == 0:
                qacc = ps_tr.tile([128, QCHUNK], f32, tag="tr", name="qacc")
            for k in range(KT):
                for sub in range(2):
                    t = 2 * th + sub
                    nc.tensor.matmul(
                        kacc[t][:],
                        lhsT=wqkv_sb[k][:, 128:256],
                        rhs=xk(0, t, k),
                        start=(k == 0), stop=(k == KT - 1),
                    )
                    nc.tensor.matmul(
                        vacc[t][:],
                        lhsT=wqkv_sb[k][:, 256:384],
                        rhs=xk(0, t, k),
                        start=(k == 0), stop=(k == KT - 1),
                    )
                if th == 0:
                    nc.tensor.matmul(
                        qacc[:],
                        lhsT=wqkv_sb[k][:, 0:128],
                        rhs=xk(0, 0, k),
                        start=(k == 0), stop=(k == KT - 1),
                    )
            # evacuate this half's projections; then its v transposes
            for sub in range(2):
                t = 2 * th + sub
                bias_evac(kacc[t], kT[0][:, t * 512:(t + 1) * 512], 1)
                bias_evac(vacc[t], vT[0][:, t * 512:(t + 1) * 512], 2)
            if th == 0:
                bias_evac(qacc, qT[0][:, 0:512], 0)
            for kt2 in range(8 * th, 8 * th + 8):
                vtrans_unit(0, kt2)

        # ---- pump generator: b0 q-chunks 1-3, then all of b1's proj ----
        def proj_chunk(b, which, t):
            ps = ps_tr.tile([128, QCHUNK], f32, tag="tr", name="projps")
            for k in range(KT):
                nc.tensor.matmul(
                    ps[:],
                    lhsT=wqkv_sb[k][:, which * 128:(which + 1) * 128],
                    rhs=xk(b, t, k),
                    start=(k == 0), stop=(k == KT - 1),
                )
                yield
            dst = (qT[b], kT[b], vT[b])[which]
            bias_evac(ps, dst[:, t * 512:(t + 1) * 512], which)
            yield
            if which == 2:
                for kt2 in range(4 * t, 4 * t + 4):
                    vtrans_unit(b, kt2)
                    yield

        # proj chunks and deferred normalize units share the single ps_tr
        # PSUM slot. A pending thunk emitted while a chunk's accumulation
        # group is open would deadlock the in-order PE queue (its WAR dep
        # waits the chunk's evac, which waits matmuls queued BEHIND it), so
        # the scheduler only runs pending thunks between chunks.
        chunks = (
            [(0, 0, t) for t in range(1, NQC)]       # b0 q chunks 1-3
            + [(1, 1, t) for t in range(NQC)]        # b1 k
            + [(1, 0, t) for t in range(NQC)]        # b1 q
            + [(1, 2, t) for t in range(NQC)]        # b1 v (+ vtrans)
        )
        sched_state = {"open": None}

        def sched_step():
            g = sched_state["open"]
            if g is not None:
                if next(g, "done") != "done":
                    return True
                sched_state["open"] = None
                return True
            if pending:
                pending.pop(0)()
                return True
            if chunks:
                g = proj_chunk(*chunks.pop(0))
                next(g, None)
                sched_state["open"] = g
                return True
            return False

        # ---- normalize units (deferred into later steps' slack) ----
        pending = []

        def _norm_unit(att_sb, og, h, qs):
            pst = ps_tr.tile([128, D + 1], f32, tag="tr", name="attt")
            nc.tensor.transpose(
                pst[:], in_=att_sb[:, qs * 128:(qs + 1) * 128], identity=id65[:]
            )
            rec = rp.tile([128, 1], f32, tag="rec", name="rec")
            nc.vector.reciprocal(out=rec[:], in_=pst[:, D:D + 1])
            nc.vector.tensor_scalar_mul(
                out=og[:, qs, h * D:(h + 1) * D], in0=pst[:, 0:D], scalar1=rec[:]
            )

        def _og_dma(og, b, qc, qs):
            eng = (nc.gpsimd, nc.sync)[qs % 2]
            eng.dma_start(
                out=out_d.rearrange("(qs p) b e -> p qs b e", p=128)[
                    :, qc * 4 + qs, b, :
                ],
                in_=og[:, qs, :],
            )

        # ---- STEADY: flat (b, qc, kt) stream, attT lagged one step ----
        lag = None          # (b, qc, kt, ex, att) awaiting its attT
        qcs = [(b, qc) for b in range(B) for qc in range(NQC)]

        def emit_attT(lg):
            lb, lqc, lkt, lex, latt = lg
            # head0: K-split row-tiled concurrent pair (kpos 0-63 / 64-127)
            nc.tensor.matmul(
                latt[0][:],
                lhsT=va[lb][0:64, lkt, 0:D + 1],
                rhs=lex[0:64, 0:QCHUNK],
                start=(lkt == 0), stop=(lkt == NKT - 1),
            )
            nc.tensor.matmul(
                latt[1][:],
                lhsT=va[lb][64:128, lkt, 0:D + 1],
                rhs=lex[64:128, 0:QCHUNK],
                start=(lkt == 0), stop=(lkt == NKT - 1),
            )
            # head1: plain M=65, K=128
            nc.tensor.matmul(
                latt[2][:],
                lhsT=va[lb][:, lkt, D + 1:VN],
                rhs=lex[:, QCHUNK:2 * QCHUNK],
                start=(lkt == 0), stop=(lkt == NKT - 1),
            )

        def finish_qc(lg):
            # evacuate accumulators -> SBUF (h0 = lo+hi fused add), queue norms
            lb, lqc, _, _, latt = lg
            og = ogp.tile([128, 4, 2 * D], f32, tag="og", name="og")
            sb0 = atn.tile([D + 1, QCHUNK], f32, tag="atn", name="attsb0")
            nc.vector.tensor_copy(out=sb0[:], in_=latt[0][:])
            nc.vector.tensor_add(out=sb0[:], in0=sb0[:], in1=latt[1][:])
            sb1 = atn.tile([D + 1, QCHUNK], f32, tag="atn", name="attsb1")
            nc.vector.tensor_copy(out=sb1[:], in_=latt[2][:])
            for qs in range(4):
                for h, sb in ((0, sb0), (1, sb1)):
                    pending.append(
                        lambda a=sb, hh=h, q=qs, o=og: _norm_unit(a, o, hh, q)
                    )
                pending.append(lambda o=og, bb=lb, qq=lqc, q=qs: _og_dma(o, bb, qq, q))

        for b, qc in qcs:
            att = [
                ps_att.tile([D + 1, QCHUNK], f32, tag="att", name=f"attps{i}")
                for i in range(3)
            ]
            qsl = qT[b][:, qc * QCHUNK:(qc + 1) * QCHUNK]
            for kt in range(NKT):
                sc = ps_sc.tile([128, 1024], f32, tag="sc", name="scps")
                for h in range(HPC):
                    nc.tensor.matmul(
                        sc[:, h * 512:(h + 1) * 512],
                        lhsT=kT[b][h * 64:(h + 1) * 64, kt * 128:(kt + 1) * 128],
                        rhs=qsl[h * 64:(h + 1) * 64, :],
                        start=True, stop=True,
                    )
                if lag is not None:
                    emit_attT(lag)
                    if lag[2] == NKT - 1:
                        finish_qc(lag)
                ex = expp.tile([128, 1024], bf, tag="ex", name="ex")
                nc.scalar.activation(
                    out=ex[:], in_=sc[:], func=mybir.ActivationFunctionType.Exp
                )
                lag = (b, qc, kt, ex, att)
                for _ in range(3):
                    sched_step()

        # drain: last attT, last evacuation, remaining chunks + pending
        emit_attT(lag)
        finish_qc(lag)
        while sched_step():
            pass

    nc.compile()
    return nc


def _get_nc():
    if "nc" not in _BUILT:
        _BUILT["nc"] = _build_bass()
    return _BUILT["nc"]


def _prep_core_inputs(x_bf, W, b):
    """Per-core input dicts. W/b slicing+scaling+casting is host-side weight prep."""
    _id128 = np.eye(128, dtype=np.float32).astype(_BF16)
    _id65 = np.eye(65, dtype=np.float32)
    in_maps = []
    for c in range(NCORES):
        q0 = 2 * c * D          # first col of this core's head pair
        wq = W[:, q0:q0 + 128] * SCALING
        wk = W[:, E + q0:E + q0 + 128]
        wv = W[:, 2 * E + q0:2 * E + q0 + 128]
        wqkv = np.concatenate([wq, wk, wv], axis=1).astype(_BF16)
        bqkv = np.concatenate(
            [b[q0:q0 + 128] * SCALING, b[E + q0:E + q0 + 128],
             b[2 * E + q0:2 * E + q0 + 128]]
        ).astype(np.float32)[:, None]
        in_maps.append(
            {
                "xt": x_bf,
                "wqkv": np.ascontiguousarray(wqkv),
                "bqkv": np.ascontiguousarray(bqkv),
                "id128": _id128,
                "id65": _id65,
            }
        )
    return in_maps


def run(inputs, trace=False):
    """Returns (output [S,B,E] fp32, BassKernelResults)."""
    from concourse.bass_utils import run_bass_kernel_spmd

    x = np.asarray(inputs["x"], np.float32)
    W = np.asarray(inputs["W_in"], np.float32)
    b = np.asarray(inputs["b_in"], np.float32)
    # sharding prep: cast + transpose to [B, TH, KT, 128, 1024]
    x_bf = np.ascontiguousarray(
        x.reshape(TH, 1024, B, KT, 128).transpose(2, 0, 3, 4, 1)
    ).astype(_BF16)

    nc = _get_nc()
    in_maps = _prep_core_inputs(x_bf, W, b)
    res = run_bass_kernel_spmd(
        nc, in_maps, core_ids=list(range(NCORES)), trace=trace
    )
    out = np.concatenate([r["out"] for r in res.results], axis=2)
    return out, res


def kernel(**inputs):
    out, _ = run(inputs, trace=False)
    return out


# revision 14
# speedup vs baseline: 1.2170x; 1.2170x over previous
"""Multi-head attention (S=2048, B=2, E=1024, H=16, D=64) on 8 Trainium2 cores.

Sharding: batch*heads head-parallel. Core c owns heads {2c, 2c+1} for both
batch elements (4 of the 32 (b,h) attention pairs). Host-side sharding prep:
slice/scale/cast the in_proj weights per core, cast x to bf16 and lay it out
as xT tiles [B, 2, KT, 128, 1024] (contraction dim on partitions, tokens
split into two halves), and concatenate the per-core outputs along E.

Pipeline (v2 — ACT-paced):
  HEAD  x(b0) DMAs stream per (token-half, k-tile); k/v projections and the
        first q-chunk accumulate round-by-round as tiles land, so the first
        exp fires right after b0's x finishes loading (~20us vs ~38us when
        projecting after the full load). A PE warm-up braid of dummy matmuls
        beats the HAM cold clock before the first projection round.
  STEADY per (b, qc, kt) step: the two heads' score matmuls go to one
        [128,1024] fp32 PSUM tile as a row-tiled concurrent pair (K=64,
        lhsT base partitions 0/64); one ScalarE Exp per kt evacuates
        PSUM->SBUF bf16. The attT (attn^T accumulation) for kt runs one kt
        LATE, emitted after the next kt's score matmuls: ScalarE never waits
        on the PE, which works through sc(kt+1) | attT(kt) | pumped-proj in
        each exp's shadow. attT itself: head0 as a K-split row-tiled pair
        (kpos 0-63 / 64-127 into separate PSUM banks, concurrent), head1
        plain M=65. Row 64 of each accumulator is sum(exp) (ones columns in
        va). b1's projections + v transposes pump into PE slack at rate/kt.
  TAIL  each finished qc defers its normalize (PE transpose back to [q,d],
        reciprocal of the denominator row, per-partition scalar multiply)
        into following steps' slack; output DMA per qs-block of 128 tokens
        on alternating queues.

PSUM budget (8 banks of 2KB): sc 2x[128,1024]f32 = 4, attT accumulators
h0lo/h0hi/h1 = 3, shared transient (warm/proj/vtrans/norm-transpose) = 1.
Measured on trn2: 227us baseline -> this version targets ~175us.
"""

import numpy as np
import ml_dtypes

S, B, E = 2048, 2, 1024
H, D = 16, 64
SCALING = D ** -0.5
NCORES = 8
HPC = H // NCORES     # 2 heads per core
KT = E // 128         # 8 contraction tiles over E
QCHUNK = 512
NQC = S // QCHUNK     # 4 q-chunks
NKT = S // 128        # 16 kpos tiles
VN = 2 * (D + 1)      # 130 va cols: [v_h0(64) | 1 | v_h1(64) | 1]
TH = 2                # token halves of 1024

_BF16 = ml_dtypes.bfloat16
_BUILT = {}


def _build_bass():
    import concourse.bacc as bacc
    import concourse.mybir as mybir
    import concourse.tile as tile
    from contextlib import ExitStack

    f32 = mybir.dt.float32
    bf = mybir.dt.bfloat16

    nc = bacc.Bacc(None, target_bir_lowering=False, debug=False)

    xt_in = nc.dram_tensor("xt", [B, TH, KT, 128, 1024], bf, kind="ExternalInput")
    wqkv_in = nc.dram_tensor("wqkv", [E, 384], bf, kind="ExternalInput")
    bqkv_in = nc.dram_tensor("bqkv", [384, 1], f32, kind="ExternalInput")
    id128_in = nc.dram_tensor("id128", [128, 128], bf, kind="ExternalInput")
    id65_in = nc.dram_tensor("id65", [65, 65], f32, kind="ExternalInput")
    out_d = nc.dram_tensor("out", [S, B, 2 * D], f32, kind="ExternalOutput")

    with tile.TileContext(nc) as tc, ExitStack() as ctx:
        const = ctx.enter_context(tc.tile_pool(name="const", bufs=1))
        res = ctx.enter_context(tc.tile_pool(name="res", bufs=1))
        expp = ctx.enter_context(tc.tile_pool(name="expp", bufs=4))
        atn = ctx.enter_context(tc.tile_pool(name="atn", bufs=4))
        ogp = ctx.enter_context(tc.tile_pool(name="ogp", bufs=8))
        rp = ctx.enter_context(tc.tile_pool(name="rp", bufs=16))
        ps_sc = ctx.enter_context(tc.tile_pool(name="ps_sc", bufs=2, space="PSUM"))
        ps_att = ctx.enter_context(tc.tile_pool(name="ps_att", bufs=2, space="PSUM"))
        ps_tr = ctx.enter_context(tc.tile_pool(name="ps_tr", bufs=2, space="PSUM"))

        # ---- constants on the gpsimd queue (sync/scalar kept free for x) ----
        wqkv_sb = [const.tile([128, 384], bf, tag=f"wqkv{k}", name=f"wqkv{k}") for k in range(KT)]
        for k in range(KT):
            nc.gpsimd.dma_start(out=wqkv_sb[k][:], in_=wqkv_in[k * 128:(k + 1) * 128, :])
        bqkv_sb = const.tile([128, 3], f32, tag="bqkv")
        nc.gpsimd.dma_start(
            out=bqkv_sb[:], in_=bqkv_in.rearrange("(c p) o -> p (c o)", p=128)
        )
        id128 = const.tile([128, 128], bf, tag="id128")
        nc.gpsimd.dma_start(out=id128[:], in_=id128_in[:])
        id65 = const.tile([65, 65], f32, tag="id65")
        nc.gpsimd.dma_start(out=id65[:], in_=id65_in[:])

        # ---- x DMAs: per (b, th, k) 256KB chunks, b0 first, 2 queues ----
        xs_sb = [
            [res.tile([128, KT, 1024], bf, tag=f"xs{b}_{t}", name=f"xs{b}_{t}") for t in range(TH)]
            for b in range(B)
        ]
        for b in range(B):
            for th in range(TH):
                for k in range(KT):
                    eng = (nc.sync, nc.scalar)[k % 2]
                    eng.dma_start(out=xs_sb[b][th][:, k, :], in_=xt_in[b, th, k])

        def xs(b, t):
            # projection chunk t (512 tokens) -> (th, sub) slices, per k below
            return xs_sb[b][t // 2]

        def xk(b, t, k):
            return xs_sb[b][t // 2][:, k, (t % 2) * 512:(t % 2) * 512 + 512]

        # ---- persistent SBUF results ----
        qT = [res.tile([128, S], bf, tag=f"qT{b}", name=f"qTt{b}") for b in range(B)]
        kT = [res.tile([128, S], bf, tag=f"kT{b}", name=f"kTt{b}") for b in range(B)]
        vT = [res.tile([128, S], bf, tag=f"vT{b}", name=f"vTt{b}") for b in range(B)]
        va = [res.tile([128, NKT, VN], bf, tag=f"va{b}", name=f"vat{b}") for b in range(B)]
        for b in range(B):
            nc.vector.memset(va[b][:], 1.0)  # ones cols survive at 64, 129

        # ---- PE warm-up braid: dummy matmuls against the HAM cold clock ----
        dm = res.tile([128, 256], bf, tag="dm")
        nc.vector.memset(dm[:], 0.125)
        for _ in range(12):
            warm = ps_tr.tile([128, 256], f32, tag="tr", name="warm")
            nc.tensor.matmul(
                warm[:], lhsT=dm[:, 0:128], rhs=dm[:], start=True, stop=True
            )

        def bias_evac(ps, dst_col_slice, which):
            nc.vector.tensor_scalar_add(
                out=dst_col_slice, in0=ps[:], scalar1=bqkv_sb[:, which:which + 1]
            )

        def vtrans_unit(b, kt2):
            # one [128,128] transpose: vT cols kt2*128.. -> va[:, kt2, d-cols]
            pst = ps_tr.tile([128, 128], bf, tag="tr", name="vtps")
            nc.tensor.transpose(
                pst[:], in_=vT[b][:, kt2 * 128:(kt2 + 1) * 128], identity=id128[:]
            )
            nc.vector.tensor_copy(
                out=va[b][:, kt2, :].rearrange("p (g x) -> p g x", g=2)[:, :, 0:64],
                in_=pst[:].rearrange("p (g d) -> p g d", g=2),
            )

        # ---- HEAD: b0 k/v/q0 projections pipelined with the x stream ----
        # accumulators: k chunks 2t+0/2t+1 and v chunks in ps_att + ps_sc
        # slots, q-chunk0 in ps_tr (after the warm braid).
        kacc = {}
        vacc = {}
        qacc = None
        for th in range(TH):
            kacc[2 * th] = ps_att.tile([128, QCHUNK], f32, tag="att", name="kacc0")
            kacc[2 * th + 1] = ps_att.tile([128, QCHUNK], f32, tag="att", name="kacc1")
            vacc[2 * th] = ps_sc.tile([128, QCHUNK], f32, tag="sc", name="vacc0")
            vacc[2 * th + 1] = ps_sc.tile([128, QCHUNK], f32, tag="sc", name="vacc1")
            if th == 0:
                qacc = ps_tr.tile([128, QCHUNK], f32, tag="tr", name="qacc")
            for k in range(KT):
                for sub in range(2):
                    t = 2 * th + sub
                    nc.tensor.matmul(
                        kacc[t][:],
                        lhsT=wqkv_sb[k][:, 128:256],
                        rhs=xk(0, t, k),
                        start=(k == 0), stop=(k == KT - 1),
                    )
                    nc.tensor.matmul(
                        vacc[t][:],
                        lhsT=wqkv_sb[k][:, 256:384],
                        rhs=xk(0, t, k),
                        start=(k == 0), stop=(k == KT - 1),
                    )
                if th == 0:
                    nc.tensor.matmul(
                        qacc[:],
                        lhsT=wqkv_sb[k][:, 0:128],
                        rhs=xk(0, 0, k),
                        start=(k == 0), stop=(k == KT - 1),
                    )
            # evacuate this half's projections; then its v transposes
            for sub in range(2):
                t = 2 * th + sub
                bias_evac(kacc[t], kT[0][:, t * 512:(t + 1) * 512], 1)
                bias_evac(vacc[t], vT[0][:, t * 512:(t + 1) * 512], 2)
            if th == 0:
                bias_evac(qacc, qT[0][:, 0:512], 0)
            for kt2 in range(8 * th, 8 * th + 8):
                vtrans_unit(0, kt2)

        # ---- pump generator: b0 q-chunks 1-3, then all of b1's proj ----
        def proj_chunk(b, which, t):
            ps = ps_tr.tile([128, QCHUNK], f32, tag="tr", name="projps")
            for k in range(KT):
                nc.tensor.matmul(
                    ps[:],
                    lhsT=wqkv_sb[k][:, which * 128:(which + 1) * 128],
                    rhs=xk(b, t, k),
                    start=(k == 0), stop=(k == KT - 1),
                )
                yield
            dst = (qT[b], kT[b], vT[b])[which]
            bias_evac(ps, dst[:, t * 512:(t + 1) * 512], which)
            yield
            if which == 2:
                for kt2 in range(4 * t, 4 * t + 4):
                    vtrans_unit(b, kt2)
                    yield

        # proj chunks and deferred normalize units share the single ps_tr
        # PSUM slot. A pending thunk emitted while a chunk's accumulation
        # group is open would deadlock the in-order PE queue (its WAR dep
        # waits the chunk's evac, which waits matmuls queued BEHIND it), so
        # the scheduler only runs pending thunks between chunks.
        chunks = (
            [(0, 0, t) for t in range(1, NQC)]       # b0 q chunks 1-3
            + [(1, 1, t) for t in range(NQC)]        # b1 k
            + [(1, 0, t) for t in range(NQC)]        # b1 q
            + [(1, 2, t) for t in range(NQC)]        # b1 v (+ vtrans)
        )
        sched_state = {"open": None}

        def sched_step():
            g = sched_state["open"]
            if g is not None:
                if next(g, "done") != "done":
                    return True
                sched_state["open"] = None
                return True
            if pending:
                pending.pop(0)()
                return True
            if chunks:
                g = proj_chunk(*chunks.pop(0))
                next(g, None)
                sched_state["open"] = g
                return True
            return False

        # ---- normalize units (deferred into later steps' slack) ----
        pending = []

        def _norm_unit(att_sb, og, h, qs):
            pst = ps_tr.tile([128, D + 1], f32, tag="tr", name="attt")
            nc.tensor.transpose(
                pst[:], in_=att_sb[:, qs * 128:(qs + 1) * 128], identity=id65[:]
            )
            rec = rp.tile([128, 1], f32, tag="rec", name="rec")
            nc.vector.reciprocal(out=rec[:], in_=pst[:, D:D + 1])
            nc.vector.tensor_scalar_mul(
                out=og[:, qs, h * D:(h + 1) * D], in0=pst[:, 0:D], scalar1=rec[:]
            )

        def _og_dma(og, b, qc, qs):
            eng = (nc.gpsimd, nc.sync)[qs % 2]
            eng.dma_start(
                out=out_d.rearrange("(qs p) b e -> p qs b e", p=128)[
                    :, qc * 4 + qs, b, :
                ],
                in_=og[:, qs, :],
            )

        # ---- STEADY: flat (b, qc, kt) stream, attT lagged one step ----
        lag = None          # (b, qc, kt, ex, att) awaiting its attT
        qcs = [(b, qc) for b in range(B) for qc in range(NQC)]

        def emit_attT(lg):
            lb, lqc, lkt, lex, latt = lg
            for h in range(HPC):
                nc.tensor.matmul(
                    latt[h][:],
                    lhsT=va[lb][:, lkt, h * (D + 1):(h + 1) * (D + 1)],
                    rhs=lex[:, h * QCHUNK:(h + 1) * QCHUNK],
                    start=(lkt == 0), stop=(lkt == NKT - 1),
                )

        def finish_qc(lg):
            # evacuate accumulators -> SBUF (h0 = lo+hi fused add), queue norms
            lb, lqc, _, _, latt = lg
            og = ogp.tile([128, 4, 2 * D], f32, tag="og", name="og")
            sb0 = atn.tile([D + 1, QCHUNK], f32, tag="atn", name="attsb0")
            nc.vector.tensor_copy(out=sb0[:], in_=latt[0][:])
            sb1 = atn.tile([D + 1, QCHUNK], f32, tag="atn", name="attsb1")
            nc.vector.tensor_copy(out=sb1[:], in_=latt[1][:])
            for qs in range(4):
                for h, sb in ((0, sb0), (1, sb1)):
                    pending.append(
                        lambda a=sb, hh=h, q=qs, o=og: _norm_unit(a, o, hh, q)
                    )
                pending.append(lambda o=og, bb=lb, qq=lqc, q=qs: _og_dma(o, bb, qq, q))

        for b, qc in qcs:
            att = [
                ps_att.tile([D + 1, QCHUNK], f32, tag="att", name=f"attps{i}")
                for i in range(HPC)
            ]
            qsl = qT[b][:, qc * QCHUNK:(qc + 1) * QCHUNK]
            for kt in range(NKT):
                sc = ps_sc.tile([128, 1024], f32, tag="sc", name="scps")
                for h in range(HPC):
                    nc.tensor.matmul(
                        sc[:, h * 512:(h + 1) * 512],
                        lhsT=kT[b][h * 64:(h + 1) * 64, kt * 128:(kt + 1) * 128],
                        rhs=qsl[h * 64:(h + 1) * 64, :],
                        start=True, stop=True,
                    )
                if lag is not None:
                    emit_attT(lag)
                    if lag[2] == NKT - 1:
                        finish_qc(lag)
                ex = expp.tile([128, 1024], bf, tag="ex", name="ex")
                nc.scalar.activation(
                    out=ex[:], in_=sc[:], func=mybir.ActivationFunctionType.Exp
                )
                lag = (b, qc, kt, ex, att)
                for _ in range(3):
                    sched_step()

        # drain: last attT, last evacuation, remaining chunks + pending
        emit_attT(lag)
        finish_qc(lag)
        while sched_step():
            pass

    nc.compile()
    return nc


def _get_nc():
    if "nc" not in _BUILT:
        _BUILT["nc"] = _build_bass()
    return _BUILT["nc"]


def _prep_core_inputs(x_bf, W, b):
    """Per-core input dicts. W/b slicing+scaling+casting is host-side weight prep."""
    _id128 = np.eye(128, dtype=np.float32).astype(_BF16)
    _id65 = np.eye(65, dtype=np.float32)
    in_maps = []
    for c in range(NCORES):
        q0 = 2 * c * D          # first col of this core's head pair
        wq = W[:, q0:q0 + 128] * SCALING
        wk = W[:, E + q0:E + q0 + 128]
        wv = W[:, 2 * E + q0:2 * E + q0 + 128]
        wqkv = np.concatenate([wq, wk, wv], axis=1).astype(_BF16)
        bqkv = np.concatenate(
            [b[q0:q0 + 128] * SCALING, b[E + q0:E + q0 + 128],
             b[2 * E + q0:2 * E + q0 + 128]]
        ).astype(np.float32)[:, None]
        in_maps.append(
            {
                "xt": x_bf,
                "wqkv": np.ascontiguousarray(wqkv),
                "bqkv": np.ascontiguousarray(bqkv),
                "id128": _id128,
                "id65": _id65,
            }
        )
    return in_maps


def run(inputs, trace=False):
    """Returns (output [S,B,E] fp32, BassKernelResults)."""
    from concourse.bass_utils import run_bass_kernel_spmd

    x = np.asarray(inputs["x"], np.float32)
    W = np.asarray(inputs["W_in"], np.float32)
    b = np.asarray(inputs["b_in"], np.float32)
    # sharding prep: cast + transpose to [B, TH, KT, 128, 1024]
    x_bf = np.ascontiguousarray(
        x.reshape(TH, 1024, B, KT, 128).transpose(2, 0, 3, 4, 1)
    ).astype(_BF16)

    nc = _get_nc()
    in_maps = _prep_core_inputs(x_bf, W, b)
    res = run_bass_kernel_spmd(
        nc, in_maps, core_ids=list(range(NCORES)), trace=trace
    )
    out = np.concatenate([r["out"] for r in res.results], axis=2)
    return out, res


def kernel(**inputs):
    out, _ = run(inputs, trace=False)
    return out


# revision 24
# speedup vs baseline: 1.2423x; 1.0208x over previous
"""Multi-head attention (S=2048, B=2, E=1024, H=16, D=64) on 8 Trainium2 cores.

Sharding: batch*heads head-parallel. Core c owns heads {2c, 2c+1} for both
batch elements (4 of the 32 (b,h) attention pairs). Host-side sharding prep:
slice/scale/cast the in_proj weights per core, cast x to bf16 and lay it out
as xT tiles [B, 2, KT, 128, 1024] (contraction dim on partitions, tokens
split into two halves), and concatenate the per-core outputs along E.

Pipeline (v2 — ACT-paced):
  HEAD  x(b0) DMAs stream per (token-half, k-tile); k/v projections and the
        first q-chunk accumulate round-by-round as tiles land, so the first
        exp fires right after b0's x finishes loading (~20us vs ~38us when
        projecting after the full load). A PE warm-up braid of dummy matmuls
        beats the HAM cold clock before the first projection round.
  STEADY per (b, qc, kt) step: the two heads' score matmuls go to one
        [128,1024] fp32 PSUM tile as a row-tiled concurrent pair (K=64,
        lhsT base partitions 0/64); one ScalarE Exp per kt evacuates
        PSUM->SBUF bf16. The attT (attn^T accumulation) for kt runs one kt
        LATE, emitted after the next kt's score matmuls: ScalarE never waits
        on the PE, which works through sc(kt+1) | attT(kt) | pumped-proj in
        each exp's shadow. attT itself: head0 as a K-split row-tiled pair
        (kpos 0-63 / 64-127 into separate PSUM banks, concurrent), head1
        plain M=65. Row 64 of each accumulator is sum(exp) (ones columns in
        va). b1's projections + v transposes pump into PE slack at rate/kt.
  TAIL  each finished qc defers its normalize (PE transpose back to [q,d],
        reciprocal of the denominator row, per-partition scalar multiply)
        into following steps' slack; output DMA per qs-block of 128 tokens
        on alternating queues.

PSUM budget (8 banks of 2KB): sc 2x[128,1024]f32 = 4, attT accumulators
h0lo/h0hi/h1 = 3, shared transient (warm/proj/vtrans/norm-transpose) = 1.
Measured on trn2: 227us baseline -> this version targets ~175us.
"""

import numpy as np
import ml_dtypes

S, B, E = 2048, 2, 1024
H, D = 16, 64
SCALING = D ** -0.5
NCORES = 8
HPC = H // NCORES     # 2 heads per core
KT = E // 128         # 8 contraction tiles over E
QCHUNK = 512
NQC = S // QCHUNK     # 4 q-chunks
NKT = S // 128        # 16 kpos tiles
VN = 2 * (D + 1)      # 130 va cols: [v_h0(64) | 1 | v_h1(64) | 1]
TH = 2                # token halves of 1024

_BF16 = ml_dtypes.bfloat16
_BUILT = {}


def _build_bass():
    import concourse.bacc as bacc
    import concourse.mybir as mybir
    import concourse.tile as tile
    from contextlib import ExitStack

    f32 = mybir.dt.float32
    bf = mybir.dt.bfloat16

    nc = bacc.Bacc(None, target_bir_lowering=False, debug=False)

    xt_in = nc.dram_tensor("xt", [B, TH, KT, 128, 1024], bf, kind="ExternalInput")
    wqkv_in = nc.dram_tensor("wqkv", [E, 384], bf, kind="ExternalInput")
    bqkv_in = nc.dram_tensor("bqkv", [384, 1], f32, kind="ExternalInput")
    id128_in = nc.dram_tensor("id128", [128, 128], bf, kind="ExternalInput")
    id65_in = nc.dram_tensor("id65", [65, 65], f32, kind="ExternalInput")
    out_d = nc.dram_tensor("out", [S, B, 2 * D], f32, kind="ExternalOutput")

    with tile.TileContext(nc) as tc, ExitStack() as ctx:
        const = ctx.enter_context(tc.tile_pool(name="const", bufs=1))
        res = ctx.enter_context(tc.tile_pool(name="res", bufs=1))
        expp = ctx.enter_context(tc.tile_pool(name="expp", bufs=4))
        atn = ctx.enter_context(tc.tile_pool(name="atn", bufs=4))
        ogp = ctx.enter_context(tc.tile_pool(name="ogp", bufs=8))
        rp = ctx.enter_context(tc.tile_pool(name="rp", bufs=16))
        ps_sc = ctx.enter_context(tc.tile_pool(name="ps_sc", bufs=2, space="PSUM"))
        ps_att = ctx.enter_context(tc.tile_pool(name="ps_att", bufs=2, space="PSUM"))
        ps_tr = ctx.enter_context(tc.tile_pool(name="ps_tr", bufs=2, space="PSUM"))

        # ---- constants on the gpsimd queue (sync/scalar kept free for x) ----
        wqkv_sb = [const.tile([128, 384], bf, tag=f"wqkv{k}", name=f"wqkv{k}") for k in range(KT)]
        for k in range(KT):
            nc.gpsimd.dma_start(out=wqkv_sb[k][:], in_=wqkv_in[k * 128:(k + 1) * 128, :])
        bqkv_sb = const.tile([128, 3], f32, tag="bqkv")
        nc.gpsimd.dma_start(
            out=bqkv_sb[:], in_=bqkv_in.rearrange("(c p) o -> p (c o)", p=128)
        )
        id128 = const.tile([128, 128], bf, tag="id128")
        nc.gpsimd.dma_start(out=id128[:], in_=id128_in[:])
        id65 = const.tile([65, 65], f32, tag="id65")
        nc.gpsimd.dma_start(out=id65[:], in_=id65_in[:])

        # ---- x DMAs: per (b, th, k) 256KB chunks, b0 first, 2 queues ----
        xs_sb = [
            [res.tile([128, KT, 1024], bf, tag=f"xs{b}_{t}", name=f"xs{b}_{t}") for t in range(TH)]
            for b in range(B)
        ]
        for b in range(B):
            for th in range(TH):
                for k in range(KT):
                    eng = (nc.sync, nc.scalar)[k % 2]
                    eng.dma_start(out=xs_sb[b][th][:, k, :], in_=xt_in[b, th, k])

        def xs(b, t):
            # projection chunk t (512 tokens) -> (th, sub) slices, per k below
            return xs_sb[b][t // 2]

        def xk(b, t, k):
            return xs_sb[b][t // 2][:, k, (t % 2) * 512:(t % 2) * 512 + 512]

        # ---- persistent SBUF results ----
        qT = [res.tile([128, S], bf, tag=f"qT{b}", name=f"qTt{b}") for b in range(B)]
        kT = [res.tile([128, S], bf, tag=f"kT{b}", name=f"kTt{b}") for b in range(B)]
        vT = [res.tile([128, S], bf, tag=f"vT{b}", name=f"vTt{b}") for b in range(B)]
        va = [res.tile([128, NKT, VN], bf, tag=f"va{b}", name=f"vat{b}") for b in range(B)]
        for b in range(B):
            nc.vector.memset(va[b][:], 1.0)  # ones cols survive at 64, 129

        # ---- PE warm-up braid: dummy matmuls against the HAM cold clock ----
        dm = res.tile([128, 256], bf, tag="dm")
        nc.vector.memset(dm[:], 0.125)
        for _ in range(12):
            warm = ps_tr.tile([128, 256], f32, tag="tr", name="warm")
            nc.tensor.matmul(
                warm[:], lhsT=dm[:, 0:128], rhs=dm[:], start=True, stop=True
            )

        def bias_evac(ps, dst_col_slice, which):
            nc.vector.tensor_scalar_add(
                out=dst_col_slice, in0=ps[:], scalar1=bqkv_sb[:, which:which + 1]
            )

        def vtrans_unit(b, kt2):
            # one [128,128] transpose: vT cols kt2*128.. -> va[:, kt2, d-cols]
            pst = ps_tr.tile([128, 128], bf, tag="tr", name="vtps")
            nc.tensor.transpose(
                pst[:], in_=vT[b][:, kt2 * 128:(kt2 + 1) * 128], identity=id128[:]
            )
            nc.vector.tensor_copy(
                out=va[b][:, kt2, :].rearrange("p (g x) -> p g x", g=2)[:, :, 0:64],
                in_=pst[:].rearrange("p (g d) -> p g d", g=2),
            )

        # ---- HEAD: b0 k/v/q0 projections pipelined with the x stream ----
        # accumulators: k chunks 2t+0/2t+1 and v chunks in ps_att + ps_sc
        # slots, q-chunk0 in ps_tr (after the warm braid).
        kacc = {}
        vacc = {}
        qacc = None
        for th in range(TH):
            kacc[2 * th] = ps_att.tile([128, QCHUNK], f32, tag="att", name="kacc0")
            kacc[2 * th + 1] = ps_att.tile([128, QCHUNK], f32, tag="att", name="kacc1")
            vacc[2 * th] = ps_sc.tile([128, QCHUNK], f32, tag="sc", name="vacc0")
            vacc[2 * th + 1] = ps_sc.tile([128, QCHUNK], f32, tag="sc", name="vacc1")
            if th == 0:
                qacc = ps_tr.tile([128, QCHUNK], f32, tag="tr", name="qacc")
            for k in range(KT):
                for sub in range(2):
                    t = 2 * th + sub
                    nc.tensor.matmul(
                        kacc[t][:],
                        lhsT=wqkv_sb[k][:, 128:256],
                        rhs=xk(0, t, k),
                        start=(k == 0), stop=(k == KT - 1),
                    )
                    nc.tensor.matmul(
                        vacc[t][:],
                        lhsT=wqkv_sb[k][:, 256:384],
                        rhs=xk(0, t, k),
                        start=(k == 0), stop=(k == KT - 1),
                    )
                if th == 0:
                    nc.tensor.matmul(
                        qacc[:],
                        lhsT=wqkv_sb[k][:, 0:128],
                        rhs=xk(0, 0, k),
                        start=(k == 0), stop=(k == KT - 1),
                    )
            # evacuate this half's projections; then its v transposes
            for sub in range(2):
                t = 2 * th + sub
                bias_evac(kacc[t], kT[0][:, t * 512:(t + 1) * 512], 1)
                bias_evac(vacc[t], vT[0][:, t * 512:(t + 1) * 512], 2)
            if th == 0:
                bias_evac(qacc, qT[0][:, 0:512], 0)
                # th0's va tiles gate attT(qc0, kt 0-7): transpose here.
                # th1's (kt 8-15) are pumped into qc0's exp shadow instead.
                for kt2 in range(8):
                    vtrans_unit(0, kt2)

        # ---- pump generator: b0 q-chunks 1-3, then all of b1's proj ----
        def proj_chunk(b, which, t):
            ps = ps_tr.tile([128, QCHUNK], f32, tag="tr", name="projps")
            for k in range(KT):
                nc.tensor.matmul(
                    ps[:],
                    lhsT=wqkv_sb[k][:, which * 128:(which + 1) * 128],
                    rhs=xk(b, t, k),
                    start=(k == 0), stop=(k == KT - 1),
                )
                yield
            dst = (qT[b], kT[b], vT[b])[which]
            bias_evac(ps, dst[:, t * 512:(t + 1) * 512], which)
            if which != 2:
                proj_emitted[(b, which)] += 1
            yield
            if which == 2:
                for kt2 in range(4 * t, 4 * t + 4):
                    vtrans_unit(b, kt2)
                    va_emitted[b] = kt2 + 1
                    yield

        # proj chunks and deferred normalize units share the single ps_tr
        # PSUM slot. A pending thunk emitted while a chunk's accumulation
        # group is open would deadlock the in-order PE queue (its WAR dep
        # waits the chunk's evac, which waits matmuls queued BEHIND it), so
        # the scheduler only runs pending thunks between chunks.
        # emission-progress trackers (Tile deps are emission-ordered: a
        # consumer emitted before its producer reads stale data)
        va_emitted = {0: 8, 1: 0}       # va[b] tiles 0..n-1 transposed
        proj_emitted = {(0, 0): 1, (0, 1): 4, (1, 0): 0, (1, 1): 0}
        # (b, which) -> chunks evacuated; q0/k of b0 done in the head

        def vtrans_th1():
            for kt2 in range(8, 16):
                vtrans_unit(0, kt2)
                va_emitted[0] = kt2 + 1
                yield

        chunks = (
            [vtrans_th1]                                          # b0 va kt 8-15
            + [lambda t=t: proj_chunk(0, 0, t) for t in range(1, NQC)]  # b0 q 1-3
            + [lambda t=t: proj_chunk(1, 1, t) for t in range(NQC)]     # b1 k
            + [lambda t=t: proj_chunk(1, 0, t) for t in range(NQC)]     # b1 q
            + [lambda t=t: proj_chunk(1, 2, t) for t in range(NQC)]     # b1 v
        )
        sched_state = {"open": None}

        def sched_step():
            g = sched_state["open"]
            if g is not None:
                if next(g, "done") != "done":
                    return True
                sched_state["open"] = None
                return True
            if pending:
                pending.pop(0)()
                return True
            if chunks:
                g = chunks.pop(0)()
                next(g, None)
                sched_state["open"] = g
                return True
            return False

        def drain_open_chunk():
            # emit the open chunk to completion (ps_tr users may not
            # interleave with it -- in-order PE queue deadlock otherwise)
            g = sched_state["open"]
            if g is not None:
                for _ in g:
                    pass
                sched_state["open"] = None

        def drain_chunks_until(cond):
            # advance CHUNKS ONLY (never pendings) until cond() holds;
            # correctness guard for emission-order deadlines
            while not cond():
                g = sched_state["open"]
                if g is None:
                    assert chunks, "chunk deadline unsatisfiable"
                    g = chunks.pop(0)()
                    sched_state["open"] = g
                if next(g, "done") == "done":
                    sched_state["open"] = None

        # ---- normalize units (deferred into later steps' slack) ----
        pending = []

        def _norm_unit(att_sb, og, h, qs):
            pst = ps_tr.tile([128, D + 1], f32, tag="tr", name="attt")
            nc.tensor.transpose(
                pst[:], in_=att_sb[:, qs * 128:(qs + 1) * 128], identity=id65[:]
            )
            rec = rp.tile([128, 1], f32, tag="rec", name="rec")
            nc.vector.reciprocal(out=rec[:], in_=pst[:, D:D + 1])
            nc.vector.tensor_scalar_mul(
                out=og[:, qs, h * D:(h + 1) * D], in0=pst[:, 0:D], scalar1=rec[:]
            )

        def _og_dma(og, b, qc, qs):
            eng = (nc.gpsimd, nc.sync)[qs % 2]
            eng.dma_start(
                out=out_d.rearrange("(qs p) b e -> p qs b e", p=128)[
                    :, qc * 4 + qs, b, :
                ],
                in_=og[:, qs, :],
            )

        # ---- STEADY: flat (b, qc, kt) stream, attT lagged one step ----
        lag = None          # (b, qc, kt, ex, att) awaiting its attT
        qcs = [(b, qc) for b in range(B) for qc in range(NQC)]

        def emit_attT(lg):
            lb, lqc, lkt, lex, latt = lg
            if va_emitted[lb] <= lkt:
                drain_chunks_until(lambda: va_emitted[lb] > lkt)
            for h in range(HPC):
                nc.tensor.matmul(
                    latt[h][:],
                    lhsT=va[lb][:, lkt, h * (D + 1):(h + 1) * (D + 1)],
                    rhs=lex[:, h * QCHUNK:(h + 1) * QCHUNK],
                    start=(lkt == 0), stop=(lkt == NKT - 1),
                )

        def finish_qc(lg):
            # evacuate accumulators -> SBUF, queue deferred norm units.
            # Guard: if older qcs' norm units have backlogged, drain them now
            # (else a later sb-tile reuse would race their un-emitted reads).
            if len(pending) > 6:
                drain_open_chunk()
                while len(pending) > 2:
                    pending.pop(0)()
            lb, lqc, _, _, latt = lg
            og = ogp.tile([128, 4, 2 * D], f32, tag="og", name="og")
            sb0 = atn.tile([D + 1, QCHUNK], f32, tag="atn", name="attsb0")
            nc.vector.tensor_copy(out=sb0[:], in_=latt[0][:])
            sb1 = atn.tile([D + 1, QCHUNK], f32, tag="atn", name="attsb1")
            nc.vector.tensor_copy(out=sb1[:], in_=latt[1][:])
            for qs in range(4):
                for h, sb in ((0, sb0), (1, sb1)):
                    pending.append(
                        lambda a=sb, hh=h, q=qs, o=og: _norm_unit(a, o, hh, q)
                    )
                pending.append(lambda o=og, bb=lb, qq=lqc, q=qs: _og_dma(o, bb, qq, q))

        for b, qc in qcs:
            # emission-order deadline: kT[b] fully and qT[b] chunk qc must be
            # emitted before this attend's score matmuls
            drain_chunks_until(
                lambda: proj_emitted[(b, 1)] == NQC and proj_emitted[(b, 0)] > qc
            )
            att = [
                ps_att.tile([D + 1, QCHUNK], f32, tag="att", name=f"attps{i}")
                for i in range(HPC)
            ]
            qsl = qT[b][:, qc * QCHUNK:(qc + 1) * QCHUNK]
            for kt in range(NKT):
                sc = ps_sc.tile([128, 1024], f32, tag="sc", name="scps")
                for h in range(HPC):
                    nc.tensor.matmul(
                        sc[:, h * 512:(h + 1) * 512],
                        lhsT=kT[b][h * 64:(h + 1) * 64, kt * 128:(kt + 1) * 128],
                        rhs=qsl[h * 64:(h + 1) * 64, :],
                        start=True, stop=True,
                    )
                if lag is not None:
                    emit_attT(lag)
                    if lag[2] == NKT - 1:
                        finish_qc(lag)
                ex = expp.tile([128, 1024], bf, tag="ex", name="ex")
                nc.scalar.activation(
                    out=ex[:], in_=sc[:], func=mybir.ActivationFunctionType.Exp
                )
                lag = (b, qc, kt, ex, att)
                rate = 3 if (chunks or sched_state["open"]) else 2
                for _ in range(rate):
                    sched_step()

        # drain: last attT, last evacuation, remaining chunks + pending
        emit_attT(lag)
        finish_qc(lag)
        while sched_step():
            pass

    nc.compile()
    return nc


def _get_nc():
    if "nc" not in _BUILT:
        _BUILT["nc"] = _build_bass()
    return _BUILT["nc"]


def _prep_core_inputs(x_bf, W, b):
    """Per-core input dicts. W/b slicing+scaling+casting is host-side weight prep."""
    _id128 = np.eye(128, dtype=np.float32).astype(_BF16)
    _id65 = np.eye(65, dtype=np.float32)
    in_maps = []
    for c in range(NCORES):
        q0 = 2 * c * D          # first col of this core's head pair
        wq = W[:, q0:q0 + 128] * SCALING
        wk = W[:, E + q0:E + q0 + 128]
        wv = W[:, 2 * E + q0:2 * E + q0 + 128]
        wqkv = np.concatenate([wq, wk, wv], axis=1).astype(_BF16)
        bqkv = np.concatenate(
            [b[q0:q0 + 128] * SCALING, b[E + q0:E + q0 + 128],
             b[2 * E + q0:2 * E + q0 + 128]]
        ).astype(np.float32)[:, None]
        in_maps.append(
            {
                "xt": x_bf,
                "wqkv": np.ascontiguousarray(wqkv),
                "bqkv": np.ascontiguousarray(bqkv),
                "id128": _id128,
                "id65": _id65,
            }
        )
    return in_maps


def run(inputs, trace=False):
    """Returns (output [S,B,E] fp32, BassKernelResults)."""
    from concourse.bass_utils import run_bass_kernel_spmd

    x = np.asarray(inputs["x"], np.float32)
    W = np.asarray(inputs["W_in"], np.float32)
    b = np.asarray(inputs["b_in"], np.float32)
    # sharding prep: cast + transpose to [B, TH, KT, 128, 1024]
    x_bf = np.ascontiguousarray(
        x.reshape(TH, 1024, B, KT, 128).transpose(2, 0, 3, 4, 1)
    ).astype(_BF16)

    nc = _get_nc()
    in_maps = _prep_core_inputs(x_bf, W, b)
    res = run_bass_kernel_spmd(
        nc, in_maps, core_ids=list(range(NCORES)), trace=trace
    )
    out = np.concatenate([r["out"] for r in res.results], axis=2)
    return out, res


def kernel(**inputs):
    out, _ = run(inputs, trace=False)
    return out
